# revision 61
# baseline (speedup 1.0000x reference)
"""Trainium2 Bass kernel for nn_Decoder (LSTM decoder + dual attention).

Sharding: data-parallel over batch B=128 across 8 NeuronCores (16 samples each).
Feature-major on-chip layouts (features on partitions, time x batch free).

Structure:
  P1 (short): load weights, fold projection matrices on-device
     (M = Wq^T Wk / sqrt(E) so the K-projection disappears under softmax;
      Wvo = Wv^T Wo^T so the attention output projection folds into V),
     compute V'/te~/beta, step-0 gates in bf16, and the first xwt window.
  P2: 256 sequential LSTM steps.  Whh is fp8e3m4 (x16, h stored /16 in bf16):
     weight loads run 2x faster than bf16 and dominate the recurrence.
     The per-step cell update (DVE/ACT chain, ~2us) leaves the PE idle, so
     filler work is interleaved into each step's emission:
       - the x @ Wih^T GEMM for the slab window 16 steps ahead (SBUF-only,
         no DRAM scratch), and
       - from step 128 on, P3 attention units for the first time block.
  P3 tail: attention + output projection for the second time block.
"""

import contextlib

import numpy as np
import ml_dtypes

B, T, E, G, NCH, SC, STG = 128, 256, 512, 2048, 128, 256, 32
NCORES = 8
PB = B // NCORES  # per-core batch = 16
EC = E // 128     # E chunks = 4
SLAB = 16         # xwt window (steps)
GRP = 4           # P3 samples per group

_cache = {}


def _build(Ts):
    import concourse.mybir as mybir
    from concourse import bacc
    from concourse.tile import TileContext

    dt = mybir.dt
    AF = mybir.ActivationFunctionType
    AX = mybir.AxisListType
    AL = mybir.AluOpType
    TB = min(128, Ts)
    NBLK = Ts // TB
    NW = Ts // SLAB
    QE = float(1.0 / np.sqrt(E))
    QE16 = QE * 16.0

    nc = bacc.Bacc(None, dynamic_dma_scratch_size=4096)

    def din(name, shape, d=dt.bfloat16):
        return nc.dram_tensor(name, shape, d, kind="ExternalInput")

    ceT_d = din("ceT", [PB, E, SC])
    teT_d = din("teT", [PB, E, STG])
    xT_d = din("xT", [NCH, Ts, PB])
    whh_d = din("whhP", [E, 16, 128], dt.float8e3)   # 16*Whh, e3m4
    whhb_d = din("whhB", [E, 16, 128])               # Whh bf16 (step-0 gates)
    wih_d = din("wihP", [NCH, 16, 128])
    gbias_d = din("gbias", [16, 128], dt.float32)
    wqc_d = din("wqc", [E, E])
    wkc_d = din("wkc", [E, E])
    wvc_d = din("wvc", [E, E])
    wocT_d = din("wocT", [E, E])
    wqt_d = din("wqt", [E, E])
    wkt_d = din("wkt", [E, E])
    wvt_d = din("wvt", [E, E])
    wotT_d = din("wotT", [E, E])
    bqc_d = din("bqc_col", [E, 1])
    bvc_d = din("bvc_col", [E, 1])
    bqt_d = din("bqt_col", [E, 1])
    bvt_d = din("bvt_col", [E, 1])
    boc_d = din("boc", [E], dt.float32)
    bot_d = din("bot", [E], dt.float32)
    outWT_d = din("outWT", [2 * E, NCH])
    outb_d = din("outb", [NCH], dt.float32)
    h0T_d = din("h0T", [E, PB])
    c0T_d = din("c0T", [E, PB], dt.float32)

    out_d = nc.dram_tensor("out", [PB, NCH, Ts], dt.float32, kind="ExternalOutput")

    with TileContext(nc) as tc, contextlib.ExitStack() as ctx:
        pp = ctx.enter_context(tc.tile_pool(name="persist", bufs=1))
        p2w = ctx.enter_context(tc.tile_pool(name="p2w", bufs=3))
        p2s = ctx.enter_context(tc.tile_pool(name="p2s", bufs=2))
        gpp = ctx.enter_context(tc.tile_pool(name="gp", bufs=2, space="PSUM"))
        ps3 = ctx.enter_context(tc.tile_pool(name="ps3", bufs=2, space="PSUM"))
        cpp = ctx.enter_context(tc.tile_pool(name="cpp", bufs=2, space="PSUM"))

        # ---- persistent tiles -------------------------------------------
        hTa = pp.tile([128, 2, Ts, PB], dt.bfloat16)
        hTb = pp.tile([128, 2, Ts, PB], dt.bfloat16)

        def hch(k):
            return (hTa, k) if k < 2 else (hTb, k - 2)

        cT = pp.tile([128, EC, PB], dt.float32)
        h0 = pp.tile([128, EC, PB], dt.bfloat16)
        ce = pp.tile([128, EC, PB, SC], dt.bfloat16)
        te = pp.tile([128, EC, PB, STG], dt.bfloat16)
        whh = pp.tile([128, EC, 16, 128], dt.float8e3)
        g0 = pp.tile([128, 2, 4, 2, PB], dt.float32)
        wih = pp.tile([128, 16, 128], dt.bfloat16)
        gb = pp.tile([128, 16], dt.float32)
        mc = pp.tile([128, EC, EC, 128], dt.bfloat16)
        bqe = pp.tile([128, EC], dt.float32)
        vcp = pp.tile([128, 2, PB, E], dt.bfloat16)
        vtp = pp.tile([128, PB, E], dt.bfloat16)
        te2 = pp.tile([128, EC, PB, STG], dt.bfloat16)
        beta = pp.tile([1, PB, STG], dt.bfloat16)
        ones1 = pp.tile([1, 128], dt.bfloat16)
        badc = pp.tile([128, EC], dt.float32)
        badt = pp.tile([128, EC], dt.float32)
        oW = pp.tile([128, 2 * EC, NCH], dt.bfloat16)
        ob = pp.tile([128, 1], dt.float32)

        nc.vector.memset(ones1[:, :], 1.0)
        # sync queue: small init + wih + first x windows
        nc.sync.dma_start(h0[:, :, :], h0T_d.rearrange("(k p) b -> p k b", p=128))
        nc.sync.dma_start(cT[:, :, :], c0T_d.rearrange("(k p) b -> p k b", p=128))
        nc.sync.dma_start(gb[:, :], gbias_d.rearrange("j p -> p j"))
        nc.sync.dma_start(ob[:, :], outb_d[:, None])
        nc.sync.dma_start(wih[:, :, :], wih_d[:, :, :])

        xbufs, slabs = {}, {}

        def load_xbuf(w):
            xb = p2s.tile([128, SLAB, PB], dt.bfloat16, tag="xbuf", bufs=3)
            nc.scalar.dma_start(xb[:, :, :],
                                xT_d[:, w * SLAB:(w + 1) * SLAB, :])
            xbufs[w] = xb

        def xwt_unit(w, j):
            # one gate tile of the x @ Wih^T slab for window w (SBUF-only)
            if j == 0:
                slabs[w] = p2s.tile([128, SLAB, 2, 4, 2, PB], dt.bfloat16,
                                    tag="slab", name=f"slab{w}")
            ps = ps3.tile([128, SLAB, PB], dt.float32, tag="pc")
            nc.tensor.matmul(ps[:, :, :], wih[:, j, :], xbufs[w][:, :, :])
            dst = slabs[w][:, :, j // 8, (j // 2) % 4, j % 2, :]
            if j % 2 == 0:
                nc.vector.tensor_scalar_add(dst, ps[:, :, :], gb[:, j:j + 1])
            else:
                nc.scalar.activation(dst, ps[:, :, :], AF.Identity,
                                     bias=gb[:, j:j + 1])

        load_xbuf(0)
        load_xbuf(1)
        # scalar (second) DMA queue: P2 weights first, then fold inputs, bulk
        for k in range(EC):
            nc.scalar.dma_start(whh[:, k, :, :], whh_d[k * 128:(k + 1) * 128])
        for k in range(EC):
            nc.scalar.dma_start(
                te[:, k, :, :],
                teT_d[:, k * 128:(k + 1) * 128, :].rearrange("i p s -> p i s"))

        # =================================================================
        # P1: on-device weight folds + V'/te~/beta/g0 + xwt window 0
        # =================================================================
        with tc.tile_pool(name="p1", bufs=1) as p1:
            for j in range(16):
                xwt_unit(0, j)

            # --- step-0 hidden gates in bf16 (h0 ~ N(0,1): too large for
            # fp8 weight noise; later h is tanh-bounded) ---
            g0ps = ps3.tile([128, 2, 4, 2, PB], dt.float32, tag="pc")
            for k in range(EC):
                whhb = p1.tile([128, 16, 128], dt.bfloat16, tag="whhb", bufs=2)
                nc.scalar.dma_start(whhb[:, :, :],
                                    whhb_d[k * 128:(k + 1) * 128])
                for h in range(2):
                    for g in range(4):
                        for e2 in range(2):
                            nc.tensor.matmul(
                                g0ps[:, h, g, e2, :],
                                whhb[:, h * 8 + g * 2 + e2, :],
                                h0[:, k, :],
                                start=(k == 0 and h == 0 and g == 0 and e2 == 0),
                                stop=(k == EC - 1 and h == 1 and g == 3
                                      and e2 == 1))
            nc.any.tensor_copy(g0[:, :, :, :, :], g0ps[:, :, :, :, :])

            wvoc = p1.tile([128, EC, E], dt.bfloat16)
            wvot = p1.tile([128, EC, E], dt.bfloat16)
            mtT = p1.tile([128, EC, E], dt.bfloat16)
            vtC = p1.tile([128, EC, 1], dt.bfloat16)
            bqcC = p1.tile([128, EC, 1], dt.bfloat16)
            bvcC = p1.tile([128, EC, 1], dt.bfloat16)
            bqtC = p1.tile([128, EC, 1], dt.bfloat16)
            bvtC = p1.tile([128, EC, 1], dt.bfloat16)
            bocS = p1.tile([128, EC], dt.float32)
            botS = p1.tile([128, EC], dt.float32)
            for (dst, src) in ((bqcC, bqc_d), (bvcC, bvc_d),
                               (bqtC, bqt_d), (bvtC, bvt_d)):
                nc.scalar.dma_start(dst[:, :, :],
                                    src.rearrange("(k p) o -> p k o", p=128))
            nc.scalar.dma_start(bocS[:, :], boc_d.rearrange("(k p) -> p k", p=128))
            nc.scalar.dma_start(botS[:, :], bot_d.rearrange("(k p) -> p k", p=128))

            def fold_phase(wL_d, wS_d):
                wL = p1.tile([128, EC, EC, 128], dt.bfloat16, tag="wL", bufs=2)
                wS = p1.tile([128, EC, E], dt.bfloat16, tag="wS", bufs=2)
                for k in range(EC):
                    nc.scalar.dma_start(
                        wL[:, k, :, :],
                        wL_d[k * 128:(k + 1) * 128, :]
                        .rearrange("p (m c) -> p m c", c=128))
                nc.scalar.dma_start(wS[:, :, :],
                                    wS_d.rearrange("(k p) e -> p k e", p=128))
                return wL, wS

            # --- phase A: M_c = Wq_c^T Wk_c (raw) ; v_c*QE -> bqe ---
            wL, wS = fold_phase(wqc_d, wkc_d)
            for m in range(EC):
                ps = cpp.tile([128, E], dt.float32, tag="cps")
                for k in range(EC):
                    nc.tensor.matmul(ps[:, :], wL[:, k, m, :], wS[:, k, :],
                                     start=(k == 0), stop=(k == EC - 1))
                for f2 in range(EC):
                    nc.any.tensor_copy(mc[:, m, f2, :],
                                       ps[:, f2 * 128:(f2 + 1) * 128])
            for m in range(EC):
                ps = ps3.tile([128, 1], dt.float32, tag="pc")
                for k in range(EC):
                    nc.tensor.matmul(ps[:, :],
                                     wS[:, k, m * 128:(m + 1) * 128],
                                     bqcC[:, k, :],
                                     start=(k == 0), stop=(k == EC - 1))
                nc.scalar.activation(bqe[:, m:m + 1], ps[:, :], AF.Identity,
                                     scale=QE)
            # --- phase B: Wvo_c ; badd_c ---
            wL, wS = fold_phase(wvc_d, wocT_d)
            for m in range(EC):
                ps = cpp.tile([128, E], dt.float32, tag="cps")
                for k in range(EC):
                    nc.tensor.matmul(ps[:, :], wL[:, k, m, :], wS[:, k, :],
                                     start=(k == 0), stop=(k == EC - 1))
                nc.any.tensor_copy(wvoc[:, m, :], ps[:, :])
            for m in range(EC):
                ps = ps3.tile([128, 1], dt.float32, tag="pc")
                for k in range(EC):
                    nc.tensor.matmul(ps[:, :],
                                     wS[:, k, m * 128:(m + 1) * 128],
                                     bvcC[:, k, :],
                                     start=(k == 0), stop=(k == EC - 1))
                nc.scalar.activation(badc[:, m:m + 1], ps[:, :], AF.Identity,
                                     bias=bocS[:, m:m + 1])
            # --- phase C: M_t^T*QE16 ; v_t*QE ---
            wL, wS = fold_phase(wkt_d, wqt_d)
            for m in range(EC):
                ps = cpp.tile([128, E], dt.float32, tag="cps")
                for k in range(EC):
                    nc.tensor.matmul(ps[:, :], wL[:, k, m, :], wS[:, k, :],
                                     start=(k == 0), stop=(k == EC - 1))
                nc.scalar.activation(mtT[:, m, :], ps[:, :], AF.Identity,
                                     scale=QE16)  # hT carries h/16
            for m in range(EC):
                ps = ps3.tile([128, 1], dt.float32, tag="pc")
                for k in range(EC):
                    nc.tensor.matmul(ps[:, :], wL[:, k, m, :], bqtC[:, k, :],
                                     start=(k == 0), stop=(k == EC - 1))
                nc.scalar.activation(vtC[:, m, :], ps[:, :], AF.Identity,
                                     scale=QE)
            # --- phase D: Wvo_t ; badd_t ---
            wL, wS = fold_phase(wvt_d, wotT_d)
            for m in range(EC):
                ps = cpp.tile([128, E], dt.float32, tag="cps")
                for k in range(EC):
                    nc.tensor.matmul(ps[:, :], wL[:, k, m, :], wS[:, k, :],
                                     start=(k == 0), stop=(k == EC - 1))
                nc.any.tensor_copy(wvot[:, m, :], ps[:, :])
            for m in range(EC):
                ps = ps3.tile([128, 1], dt.float32, tag="pc")
                for k in range(EC):
                    nc.tensor.matmul(ps[:, :],
                                     wS[:, k, m * 128:(m + 1) * 128],
                                     bvtC[:, k, :],
                                     start=(k == 0), stop=(k == EC - 1))
                nc.scalar.activation(badt[:, m:m + 1], ps[:, :], AF.Identity,
                                     bias=botS[:, m:m + 1])

            # ce (bulk) + oW now on the scalar queue
            for k in range(EC):
                nc.scalar.dma_start(
                    ce[:, k, :, :],
                    ceT_d[:, k * 128:(k + 1) * 128, :]
                    .rearrange("i p s -> p i s"))
            nc.scalar.dma_start(oW[:, :, :],
                                outWT_d.rearrange("(k p) n -> p k n", p=128))

            # --- te~^T[e, i, s] = M_t te^T ; beta[i, s] = v_t . te ---
            for m in range(EC):
                ps = cpp.tile([128, PB * STG], dt.float32, tag="cps")
                for k in range(EC):
                    nc.tensor.matmul(
                        ps[:, :], mtT[:, k, m * 128:(m + 1) * 128],
                        te[:, k, :, :].rearrange("p i s -> p (i s)"),
                        start=(k == 0), stop=(k == EC - 1))
                nc.any.tensor_copy(te2[:, m, :, :].rearrange("p i s -> p (i s)"),
                                   ps[:, :])
            bps = ps3.tile([1, PB * STG], dt.float32, tag="pc")
            for k in range(EC):
                nc.tensor.matmul(bps[:, :], vtC[:, k, :],
                                 te[:, k, :, :].rearrange("p i s -> p (i s)"),
                                 start=(k == 0), stop=(k == EC - 1))
            nc.any.tensor_copy(beta[:, :, :].rearrange("o i s -> o (i s)"),
                               bps[:, :])
            # --- V'_tag (replicated to 4 partition strips) ---
            for i in range(PB):
                ps = cpp.tile([STG, E], dt.float32, tag="cps")
                for k in range(EC):
                    nc.tensor.matmul(ps[:, :], te[:, k, i, :], wvot[:, k, :],
                                     start=(k == 0), stop=(k == EC - 1))
                nc.any.tensor_copy(vtp[:STG, i, :], ps[:, :])
            for di in range(1, 4):
                nc.sync.dma_start(vtp[di * STG:(di + 1) * STG, :, :],
                                  vtp[0:STG, :, :])
            # --- V'_char[s, i, g] ---
            for i in range(PB):
                for sc in range(SC // 128):
                    ps = cpp.tile([128, E], dt.float32, tag="cps")
                    for k in range(EC):
                        nc.tensor.matmul(
                            ps[:, :], ce[:, k, i, sc * 128:(sc + 1) * 128],
                            wvoc[:, k, :],
                            start=(k == 0), stop=(k == EC - 1))
                    nc.any.tensor_copy(vcp[:, sc, i, :], ps[:, :])

        # =================================================================
        # P3 unit builder (used interleaved for block 0, serial for block 1)
        # =================================================================
        p3 = ctx.enter_context(tc.tile_pool(name="p3", bufs=2))
        gtiles = {}

        def p3_group_units(blk, grp):
            t0 = blk * TB
            i0 = grp * GRP

            def u_qproj(m0):
                def f():
                    if m0 == 0:
                        qTg = p3.tile([128, EC, GRP, TB], dt.bfloat16,
                                      tag="qT", name="qTg")
                        org = p3.tile([128, 2 * EC, GRP, TB], dt.bfloat16,
                                      tag="og", name="org")
                        gtiles[(blk, grp)] = (qTg, org)
                    qTg, org = gtiles[(blk, grp)]
                    for m in (m0, m0 + 1):
                        ps = cpp.tile([128, TB, GRP], dt.float32, tag="cps")
                        for k in range(EC):
                            hk, ks = hch(k)
                            nc.tensor.matmul(
                                ps[:, :, :], mc[:, k, m, :],
                                hk[:, ks, t0:t0 + TB, i0:i0 + GRP],
                                start=(k == 0), stop=(k == EC - 1))
                        nc.vector.tensor_scalar(
                            qTg[:, m, :, :].rearrange("p i t -> p t i"),
                            ps[:, :, :], QE16, bqe[:, m:m + 1],
                            op0=AL.mult, op1=AL.add)
                return f

            def u_tag():
                qTg, org = gtiles[(blk, grp)]
                ptp = ps3.tile([128, GRP, STG], dt.float32, tag="pc")
                for di in range(GRP):
                    for k in range(EC):
                        hk, ks = hch(k)
                        nc.tensor.matmul(
                            ptp[:, di, :], hk[:, ks, t0:t0 + TB, i0 + di],
                            te2[:, k, i0 + di, :],
                            start=(di == 0 and k == 0), stop=False,
                            skip_group_check=True)
                nc.tensor.matmul(
                    ptp[:, :, :].rearrange("p i s -> p (i s)"),
                    ones1[:, :],
                    beta[:, i0:i0 + GRP, :].rearrange("o i s -> o (i s)"),
                    start=False, stop=True, skip_group_check=True)
                pte = p3.tile([128, GRP, STG], dt.bfloat16, tag="pte")
                nc.scalar.activation(pte[:, :, :], ptp[:, :, :], AF.Exp)
                tsum = p3.tile([128, GRP], dt.float32, tag="tsum")
                nc.vector.reduce_sum(tsum[:, :], pte[:, :, :], axis=AX.X)
                trec = p3.tile([128, GRP], dt.float32, tag="trec")
                nc.vector.reciprocal(trec[:, :], tsum[:, :])
                ptn = p3.tile([128, GRP, STG], dt.bfloat16, tag="ptn")
                for di in range(GRP):
                    nc.vector.tensor_scalar_mul(ptn[:, di, :], pte[:, di, :],
                                                trec[:, di:di + 1])
                ptT = p3.tile([128, TB], dt.bfloat16, tag="ptT")
                nc.sync.dma_start_transpose(
                    ptT[:, :], ptn[:, :, :].rearrange("p i s -> p (i s)"))
                gtiles[(blk, grp, "ptT")] = ptT

            def u_score(di):
                def f():
                    qTg, org = gtiles[(blk, grp)]
                    i = i0 + di
                    pc = ps3.tile([128, SC], dt.float32, tag="pc")
                    for k in range(EC):
                        nc.tensor.matmul(pc[:, :], qTg[:, k, di, :],
                                         ce[:, k, i, :],
                                         start=(k == 0), stop=(k == EC - 1))
                    pe = p3.tile([128, SC], dt.bfloat16, tag="pe")
                    dsum = p3.tile([128, 1], dt.float32, tag="dsum")
                    nc.scalar.activation(pe[:, :], pc[:, :], AF.Exp,
                                         accum_out=dsum[:, :])
                    drec = p3.tile([128, 1], dt.float32, tag="drec")
                    nc.vector.reciprocal(drec[:, :], dsum[:, :])
                    pn = p3.tile([128, SC], dt.bfloat16, tag="pn")
                    nc.vector.tensor_scalar_mul(pn[:, :], pe[:, :],
                                                drec[:, 0:1])
                    pTt = p3.tile([128, 2, TB], dt.bfloat16, tag="pTt")
                    for sc in range(2):
                        nc.sync.dma_start_transpose(
                            pTt[:, sc, :], pn[:, sc * 128:(sc + 1) * 128])
                    gtiles[(blk, grp, di)] = pTt
                return f

            def u_ctx(di):
                def f():
                    qTg, org = gtiles[(blk, grp)]
                    pTt = gtiles[(blk, grp, di)]
                    ptT = gtiles[(blk, grp, "ptT")]
                    i = i0 + di
                    cps = cpp.tile([128, EC, TB], dt.float32, tag="cps")
                    for m in range(EC):
                        for sc in range(2):
                            nc.tensor.matmul(
                                cps[:, m, :],
                                vcp[:, sc, i, m * 128:(m + 1) * 128],
                                pTt[:, sc, :],
                                start=(m == 0 and sc == 0),
                                stop=(m == EC - 1 and sc == 1))
                    for m in range(EC):
                        nc.vector.tensor_scalar(
                            org[:, m, di, :], cps[:, m, :],
                            badc[:, m:m + 1], 0.0, op0=AL.add, op1=AL.max)
                    cp2 = cpp.tile([128, EC, TB], dt.float32, tag="cps")
                    for m in range(EC):
                        nc.tensor.matmul(
                            cp2[:, m, :],
                            vtp[di * STG:(di + 1) * STG, i,
                                m * 128:(m + 1) * 128],
                            ptT[di * STG:(di + 1) * STG, :],
                            start=(m == 0), stop=(m == EC - 1),
                            tile_position=(di * STG, 0))
                    for m in range(EC):
                        nc.vector.tensor_scalar(
                            org[:, EC + m, di, :], cp2[:, m, :],
                            badt[:, m:m + 1], 0.0, op0=AL.add, op1=AL.max)
                return f

            def u_out():
                qTg, org = gtiles[(blk, grp)]
                ps = cpp.tile([128, GRP, TB], dt.float32, tag="cps")
                for kk in range(2 * EC):
                    nc.tensor.matmul(
                        ps[:, :, :].rearrange("p i t -> p (i t)"), oW[:, kk, :],
                        org[:, kk, :, :].rearrange("p i t -> p (i t)"),
                        start=(kk == 0), stop=(kk == 2 * EC - 1))
                of = p3.tile([128, GRP, TB], dt.float32, tag="of")
                nc.vector.tensor_scalar_add(
                    of[:, :, :].rearrange("p i t -> p (i t)"),
                    ps[:, :, :].rearrange("p i t -> p (i t)"), ob[:, 0:1])
                nc.sync.dma_start(
                    out_d[i0:i0 + GRP, :, t0:t0 + TB]
                    .rearrange("i n t -> n i t"),
                    of[:, :, :])

            units = [u_qproj(0), u_qproj(2), u_tag]
            for di in range(GRP):
                units.append(u_score(di))
                units.append(u_ctx(di))
            units.append(u_out)
            return units

        def p3_block_tail(blk):
            # pairwise group interleave: hides the DMA-transpose latency
            # between a sample's score and ctx units while keeping only two
            # groups' tiles live (pool bufs=2)
            for gp0 in range(0, PB // GRP, 2):
                ua = p3_group_units(blk, gp0)
                ub = p3_group_units(blk, gp0 + 1)
                for a, b in zip(ua, ub):
                    a()
                    b()

        # =================================================================
        # P2: sequential LSTM with interleaved filler work
        # =================================================================
        fillq = []
        for grp in range(PB // GRP):
            fillq.extend(p3_group_units(0, grp))

        for t in range(Ts):
            if t % SLAB == 0:
                v = t // SLAB
                if v + 2 < NW:
                    load_xbuf(v + 2)
            slab = slabs[t // SLAB]
            for h in range(2):
                gph = None
                if t > 0:
                    gph = gpp.tile([128, 4, 2, PB], dt.float32, tag=f"gp{h}")
                    for k in range(EC):
                        hk, ks = hch(k)
                        rhs = hk[:, ks, t - 1, :]
                        for g in range(4):
                            for e2 in range(2):
                                nc.tensor.matmul(
                                    gph[:, g, e2, :],
                                    whh[:, k, h * 8 + g * 2 + e2, :], rhs,
                                    start=(k == 0 and g == 0 and e2 == 0),
                                    stop=(k == EC - 1 and g == 3 and e2 == 1))
                gsrc = gph if t > 0 else g0[:, h]
                ga = p2w.tile([128, 4, 2, PB], dt.float32, tag=f"ga{h}")
                nc.vector.tensor_add(ga[:, :, :, :], gsrc[:, :, :, :],
                                     slab[:, t % SLAB, h, :, :, :])
                # g-gate rows are pre-scaled x2 host-side, so one sigmoid
                # covers all four gates: tanh(g) = 2*sigmoid(2g) - 1
                sio = p2w.tile([128, 4, 2, PB], dt.float32, tag=f"sio{h}")
                nc.scalar.activation(sio[:, :, :, :], ga[:, :, :, :],
                                     AF.Sigmoid)
                v_ = p2w.tile([128, 2, PB], dt.float32, tag=f"v{h}")
                a_ = p2w.tile([128, 2, PB], dt.float32, tag=f"a{h}")
                nc.vector.tensor_mul(v_[:, :, :], sio[:, 1, :, :],
                                     cT[:, 2 * h:2 * h + 2, :])
                # a = (sig(2g) - 0.5) * sig(i) = i*tanh(g)/2
                nc.vector.scalar_tensor_tensor(
                    a_[:, :, :], sio[:, 3, :, :], 0.5, sio[:, 0, :, :],
                    op0=AL.subtract, op1=AL.mult)
                nc.vector.scalar_tensor_tensor(
                    cT[:, 2 * h:2 * h + 2, :], a_[:, :, :], 2.0, v_[:, :, :],
                    op0=AL.mult, op1=AL.add)
                tcc = p2w.tile([128, 2, PB], dt.float32, tag=f"tcc{h}")
                nc.scalar.activation(tcc[:, :, :],
                                     cT[:, 2 * h:2 * h + 2, :], AF.Tanh)
                # hT stores h/16 (compensates the 16x fp8 Whh scaling)
                nc.vector.scalar_tensor_tensor(
                    (hTa if h == 0 else hTb)[:, :, t, :], sio[:, 2, :, :],
                    1.0 / 16.0, tcc[:, :, :], op0=AL.mult, op1=AL.mult)
            # filler: next slab window's XWT tile; P3 block-0 units once
            # their h block is complete
            if t < Ts - SLAB:
                xwt_unit(t // SLAB + 1, t % SLAB)
            if t >= TB + 2 and fillq:
                fillq.pop(0)()

        while fillq:
            fillq.pop(0)()
        for blk in range(1, NBLK):
            p3_block_tail(blk)

    nc.compile()
    return nc


def _prep_core(inputs, core, Ts=T):
    bf = ml_dtypes.bfloat16
    f8 = ml_dtypes.float8_e3m4
    s = slice(core * PB, (core + 1) * PB)
    ce = inputs["char_encoding"][s]
    teg = inputs["tag_encoding"][s]
    tos = inputs["true_output_seq"][s][:, :Ts]
    xs = np.concatenate(
        [np.zeros((PB, 1, NCH), np.float32), tos[:, 1:, :]], axis=1)
    # Whh/Wih rows: torch gate order (i,f,g,o) -> (i,f,o,g); feature chunk
    # ec split as (half, e2); tile j = half*8 + gt*2 + e2.
    # g-gate rows x2: tanh(g) computed as 2*sigmoid(2g) - 1 on device
    W = inputs["lstm_Whh"].reshape(4, 4, 128, E)[[0, 1, 3, 2]].copy()
    W[3] *= 2.0
    whhP = W.reshape(4, 2, 2, 128, E).transpose(4, 1, 0, 2, 3).reshape(E, 16, 128)
    V = inputs["lstm_Wih"].reshape(4, 4, 128, NCH)[[0, 1, 3, 2]].copy()
    V[3] *= 2.0
    wihP = V.reshape(4, 2, 2, 128, NCH).transpose(4, 1, 0, 2, 3).reshape(NCH, 16, 128)
    gbv = (inputs["lstm_bih"] + inputs["lstm_bhh"]).reshape(4, 4, 128)[[0, 1, 3, 2]].copy()
    gbv[3] *= 2.0
    gbias = gbv.reshape(4, 2, 2, 128).transpose(1, 0, 2, 3).reshape(16, 128)
    m = {
        "ceT": np.ascontiguousarray(ce.transpose(0, 2, 1)).astype(bf),
        "teT": np.ascontiguousarray(teg.transpose(0, 2, 1)).astype(bf),
        "xT": np.ascontiguousarray(xs.transpose(2, 1, 0)).astype(bf),
        "whhP": np.ascontiguousarray(whhP * 16.0).astype(f8),
        "whhB": np.ascontiguousarray(whhP).astype(bf),
        "wihP": np.ascontiguousarray(wihP).astype(bf),
        "gbias": np.ascontiguousarray(gbias).astype(np.float32),
        "wqc": inputs["ca_Wq"].astype(bf),
        "wkc": inputs["ca_Wk"].astype(bf),
        "wvc": inputs["ca_Wv"].astype(bf),
        "wocT": np.ascontiguousarray(inputs["ca_Wo"].T).astype(bf),
        "wqt": inputs["ta_Wq"].astype(bf),
        "wkt": inputs["ta_Wk"].astype(bf),
        "wvt": inputs["ta_Wv"].astype(bf),
        "wotT": np.ascontiguousarray(inputs["ta_Wo"].T).astype(bf),
        "bqc_col": inputs["ca_bq"][:, None].astype(bf),
        "bvc_col": inputs["ca_bv"][:, None].astype(bf),
        "bqt_col": inputs["ta_bq"][:, None].astype(bf),
        "bvt_col": inputs["ta_bv"][:, None].astype(bf),
        "boc": inputs["ca_bo"].astype(np.float32),
        "bot": inputs["ta_bo"].astype(np.float32),
        "outWT": np.ascontiguousarray(inputs["out_W"].T).astype(bf),
        "outb": inputs["out_b"].astype(np.float32),
        "h0T": np.ascontiguousarray(
            np.concatenate([inputs["char_hn"][0][s],
                            inputs["char_hn"][1][s]], -1).T).astype(bf),
        "c0T": np.ascontiguousarray(
            np.concatenate([inputs["char_cn"][0][s],
                            inputs["char_cn"][1][s]], -1).T).astype(np.float32),
    }
    return m


def kernel(**inputs):
    from concourse.bass_utils import run_bass_kernel_spmd

    inputs = {k: np.asarray(v, dtype=np.float32) for k, v in inputs.items()}
    if "nc" not in _cache:
        _cache["nc"] = _build(T)
    nc = _cache["nc"]
    in_maps = [_prep_core(inputs, c) for c in range(NCORES)]
    res = run_bass_kernel_spmd(nc, in_maps, list(range(NCORES)))
    _cache["last_res"] = res
    outs = [np.asarray(res.results[c]["out"]).transpose(0, 2, 1)
            for c in range(NCORES)]
    return np.ascontiguousarray(np.concatenate(outs, axis=0)).astype(np.float32)


# revision 62
# speedup vs baseline: 1.0018x; 1.0018x over previous
"""Trainium2 Bass kernel for nn_Decoder (LSTM decoder + dual attention).

Sharding: data-parallel over batch B=128 across 8 NeuronCores (16 samples each).
Feature-major on-chip layouts (features on partitions, time x batch free).

Structure:
  P1 (short): load weights, fold projection matrices on-device
     (M = Wq^T Wk / sqrt(E) so the K-projection disappears under softmax;
      Wvo = Wv^T Wo^T so the attention output projection folds into V),
     compute V'/te~/beta, step-0 gates in bf16, and the first xwt window.
  P2: 256 sequential LSTM steps.  Whh is fp8e3m4 (x16, h stored /16 in bf16):
     weight loads run 2x faster than bf16 and dominate the recurrence.
     The per-step cell update (DVE/ACT chain, ~2us) leaves the PE idle, so
     filler work is interleaved into each step's emission:
       - the x @ Wih^T GEMM for the slab window 16 steps ahead (SBUF-only,
         no DRAM scratch), and
       - from step 128 on, P3 attention units for the first time block.
  P3 tail: attention + output projection for the second time block.
"""

import contextlib

import numpy as np
import ml_dtypes

B, T, E, G, NCH, SC, STG = 128, 256, 512, 2048, 128, 256, 32
NCORES = 8
PB = B // NCORES  # per-core batch = 16
EC = E // 128     # E chunks = 4
SLAB = 16         # xwt window (steps)
GRP = 4           # P3 samples per group

_cache = {}


def _build(Ts):
    import concourse.mybir as mybir
    from concourse import bacc
    from concourse.tile import TileContext

    dt = mybir.dt
    AF = mybir.ActivationFunctionType
    AX = mybir.AxisListType
    AL = mybir.AluOpType
    TB = min(128, Ts)
    NBLK = Ts // TB
    NW = Ts // SLAB
    QE = float(1.0 / np.sqrt(E))
    QE16 = QE * 16.0

    nc = bacc.Bacc(None, dynamic_dma_scratch_size=4096)

    def din(name, shape, d=dt.bfloat16):
        return nc.dram_tensor(name, shape, d, kind="ExternalInput")

    ceT_d = din("ceT", [PB, E, SC])
    teT_d = din("teT", [PB, E, STG])
    xT_d = din("xT", [NCH, Ts, PB])
    whh_d = din("whhP", [E, 16, 128], dt.float8e3)   # 16*Whh, e3m4
    whhb_d = din("whhB", [E, 16, 128])               # Whh bf16 (step-0 gates)
    wih_d = din("wihP", [NCH, 16, 128])
    gbias_d = din("gbias", [16, 128], dt.float32)
    wqc_d = din("wqc", [E, E])
    wkc_d = din("wkc", [E, E])
    wvc_d = din("wvc", [E, E])
    wocT_d = din("wocT", [E, E])
    wqt_d = din("wqt", [E, E])
    wkt_d = din("wkt", [E, E])
    wvt_d = din("wvt", [E, E])
    wotT_d = din("wotT", [E, E])
    bqc_d = din("bqc_col", [E, 1])
    bvc_d = din("bvc_col", [E, 1])
    bqt_d = din("bqt_col", [E, 1])
    bvt_d = din("bvt_col", [E, 1])
    boc_d = din("boc", [E], dt.float32)
    bot_d = din("bot", [E], dt.float32)
    outWT_d = din("outWT", [2 * E, NCH])
    outb_d = din("outb", [NCH], dt.float32)
    h0T_d = din("h0T", [E, PB])
    c0T_d = din("c0T", [E, PB], dt.float32)

    out_d = nc.dram_tensor("out", [PB, NCH, Ts], dt.float32, kind="ExternalOutput")

    with TileContext(nc) as tc, contextlib.ExitStack() as ctx:
        pp = ctx.enter_context(tc.tile_pool(name="persist", bufs=1))
        p2w = ctx.enter_context(tc.tile_pool(name="p2w", bufs=3))
        p2s = ctx.enter_context(tc.tile_pool(name="p2s", bufs=2))
        gpp = ctx.enter_context(tc.tile_pool(name="gp", bufs=2, space="PSUM"))
        ps3 = ctx.enter_context(tc.tile_pool(name="ps3", bufs=2, space="PSUM"))
        cpp = ctx.enter_context(tc.tile_pool(name="cpp", bufs=2, space="PSUM"))

        # ---- persistent tiles -------------------------------------------
        hTa = pp.tile([128, 2, Ts, PB], dt.bfloat16)
        hTb = pp.tile([128, 2, Ts, PB], dt.bfloat16)

        def hch(k):
            return (hTa, k) if k < 2 else (hTb, k - 2)

        cT = pp.tile([128, EC, PB], dt.float32)
        h0 = pp.tile([128, EC, PB], dt.bfloat16)
        ce = pp.tile([128, EC, PB, SC], dt.bfloat16)
        te = pp.tile([128, EC, PB, STG], dt.bfloat16)
        whh = pp.tile([128, EC, 16, 128], dt.float8e3)
        g0 = pp.tile([128, 2, 4, 2, PB], dt.float32)
        wih = pp.tile([128, 16, 128], dt.bfloat16)
        gb = pp.tile([128, 16], dt.float32)
        mc = pp.tile([128, EC, EC, 128], dt.bfloat16)
        bqe = pp.tile([128, EC], dt.float32)
        vcp = pp.tile([128, 2, PB, E], dt.bfloat16)
        vtp = pp.tile([128, PB, E], dt.bfloat16)
        te2 = pp.tile([128, EC, PB, STG], dt.bfloat16)
        beta = pp.tile([1, PB, STG], dt.bfloat16)
        ones1 = pp.tile([1, 128], dt.bfloat16)
        badc = pp.tile([128, EC], dt.float32)
        badt = pp.tile([128, EC], dt.float32)
        oW = pp.tile([128, 2 * EC, NCH], dt.bfloat16)
        ob = pp.tile([128, 1], dt.float32)

        nc.vector.memset(ones1[:, :], 1.0)
        # sync queue: small init + wih + first x windows
        nc.sync.dma_start(h0[:, :, :], h0T_d.rearrange("(k p) b -> p k b", p=128))
        nc.sync.dma_start(cT[:, :, :], c0T_d.rearrange("(k p) b -> p k b", p=128))
        nc.sync.dma_start(gb[:, :], gbias_d.rearrange("j p -> p j"))
        nc.sync.dma_start(ob[:, :], outb_d[:, None])
        nc.sync.dma_start(wih[:, :, :], wih_d[:, :, :])

        xbufs, slabs = {}, {}

        def load_xbuf(w):
            xb = p2s.tile([128, SLAB, PB], dt.bfloat16, tag="xbuf", bufs=3)
            nc.scalar.dma_start(xb[:, :, :],
                                xT_d[:, w * SLAB:(w + 1) * SLAB, :])
            xbufs[w] = xb

        def xwt_unit(w, j):
            # one gate tile of the x @ Wih^T slab for window w (SBUF-only)
            if j == 0:
                slabs[w] = p2s.tile([128, SLAB, 2, 4, 2, PB], dt.bfloat16,
                                    tag="slab", name=f"slab{w}")
            ps = ps3.tile([128, SLAB, PB], dt.float32, tag="pc")
            nc.tensor.matmul(ps[:, :, :], wih[:, j, :], xbufs[w][:, :, :])
            dst = slabs[w][:, :, j // 8, (j // 2) % 4, j % 2, :]
            if j % 2 == 0:
                nc.vector.tensor_scalar_add(dst, ps[:, :, :], gb[:, j:j + 1])
            else:
                nc.scalar.activation(dst, ps[:, :, :], AF.Identity,
                                     bias=gb[:, j:j + 1])

        load_xbuf(0)
        load_xbuf(1)
        # scalar (second) DMA queue: P2 weights first, then fold inputs, bulk
        for k in range(EC):
            nc.scalar.dma_start(whh[:, k, :, :], whh_d[k * 128:(k + 1) * 128])
        for k in range(EC):
            nc.scalar.dma_start(
                te[:, k, :, :],
                teT_d[:, k * 128:(k + 1) * 128, :].rearrange("i p s -> p i s"))

        # =================================================================
        # P1: on-device weight folds + V'/te~/beta/g0 + xwt window 0
        # =================================================================
        with tc.tile_pool(name="p1", bufs=1) as p1:
            for j in range(16):
                xwt_unit(0, j)

            wvoc = p1.tile([128, EC, E], dt.bfloat16)
            wvot = p1.tile([128, EC, E], dt.bfloat16)
            mtT = p1.tile([128, EC, E], dt.bfloat16)
            vtC = p1.tile([128, EC, 1], dt.bfloat16)
            bqcC = p1.tile([128, EC, 1], dt.bfloat16)
            bvcC = p1.tile([128, EC, 1], dt.bfloat16)
            bqtC = p1.tile([128, EC, 1], dt.bfloat16)
            bvtC = p1.tile([128, EC, 1], dt.bfloat16)
            bocS = p1.tile([128, EC], dt.float32)
            botS = p1.tile([128, EC], dt.float32)
            for (dst, src) in ((bqcC, bqc_d), (bvcC, bvc_d),
                               (bqtC, bqt_d), (bvtC, bvt_d)):
                nc.scalar.dma_start(dst[:, :, :],
                                    src.rearrange("(k p) o -> p k o", p=128))
            nc.scalar.dma_start(bocS[:, :], boc_d.rearrange("(k p) -> p k", p=128))
            nc.scalar.dma_start(botS[:, :], bot_d.rearrange("(k p) -> p k", p=128))

            def fold_phase(wL_d, wS_d):
                wL = p1.tile([128, EC, EC, 128], dt.bfloat16, tag="wL", bufs=2)
                wS = p1.tile([128, EC, E], dt.bfloat16, tag="wS", bufs=2)
                for k in range(EC):
                    nc.scalar.dma_start(
                        wL[:, k, :, :],
                        wL_d[k * 128:(k + 1) * 128, :]
                        .rearrange("p (m c) -> p m c", c=128))
                nc.scalar.dma_start(wS[:, :, :],
                                    wS_d.rearrange("(k p) e -> p k e", p=128))
                return wL, wS

            # --- phase A: M_c = Wq_c^T Wk_c (raw) ; v_c*QE -> bqe ---
            wL, wS = fold_phase(wqc_d, wkc_d)
            for m in range(EC):
                ps = cpp.tile([128, E], dt.float32, tag="cps")
                for k in range(EC):
                    nc.tensor.matmul(ps[:, :], wL[:, k, m, :], wS[:, k, :],
                                     start=(k == 0), stop=(k == EC - 1))
                for f2 in range(EC):
                    nc.any.tensor_copy(mc[:, m, f2, :],
                                       ps[:, f2 * 128:(f2 + 1) * 128])
            for m in range(EC):
                ps = ps3.tile([128, 1], dt.float32, tag="pc")
                for k in range(EC):
                    nc.tensor.matmul(ps[:, :],
                                     wS[:, k, m * 128:(m + 1) * 128],
                                     bqcC[:, k, :],
                                     start=(k == 0), stop=(k == EC - 1))
                nc.scalar.activation(bqe[:, m:m + 1], ps[:, :], AF.Identity,
                                     scale=QE)
            # --- phase B: Wvo_c ; badd_c ---
            wL, wS = fold_phase(wvc_d, wocT_d)
            for m in range(EC):
                ps = cpp.tile([128, E], dt.float32, tag="cps")
                for k in range(EC):
                    nc.tensor.matmul(ps[:, :], wL[:, k, m, :], wS[:, k, :],
                                     start=(k == 0), stop=(k == EC - 1))
                nc.any.tensor_copy(wvoc[:, m, :], ps[:, :])
            for m in range(EC):
                ps = ps3.tile([128, 1], dt.float32, tag="pc")
                for k in range(EC):
                    nc.tensor.matmul(ps[:, :],
                                     wS[:, k, m * 128:(m + 1) * 128],
                                     bvcC[:, k, :],
                                     start=(k == 0), stop=(k == EC - 1))
                nc.scalar.activation(badc[:, m:m + 1], ps[:, :], AF.Identity,
                                     bias=bocS[:, m:m + 1])
            # --- phase C: M_t^T*QE16 ; v_t*QE ---
            wL, wS = fold_phase(wkt_d, wqt_d)
            for m in range(EC):
                ps = cpp.tile([128, E], dt.float32, tag="cps")
                for k in range(EC):
                    nc.tensor.matmul(ps[:, :], wL[:, k, m, :], wS[:, k, :],
                                     start=(k == 0), stop=(k == EC - 1))
                nc.scalar.activation(mtT[:, m, :], ps[:, :], AF.Identity,
                                     scale=QE16)  # hT carries h/16
            for m in range(EC):
                ps = ps3.tile([128, 1], dt.float32, tag="pc")
                for k in range(EC):
                    nc.tensor.matmul(ps[:, :], wL[:, k, m, :], bqtC[:, k, :],
                                     start=(k == 0), stop=(k == EC - 1))
                nc.scalar.activation(vtC[:, m, :], ps[:, :], AF.Identity,
                                     scale=QE)
            # --- phase D: Wvo_t ; badd_t ---
            wL, wS = fold_phase(wvt_d, wotT_d)
            for m in range(EC):
                ps = cpp.tile([128, E], dt.float32, tag="cps")
                for k in range(EC):
                    nc.tensor.matmul(ps[:, :], wL[:, k, m, :], wS[:, k, :],
                                     start=(k == 0), stop=(k == EC - 1))
                nc.any.tensor_copy(wvot[:, m, :], ps[:, :])
            for m in range(EC):
                ps = ps3.tile([128, 1], dt.float32, tag="pc")
                for k in range(EC):
                    nc.tensor.matmul(ps[:, :],
                                     wS[:, k, m * 128:(m + 1) * 128],
                                     bvtC[:, k, :],
                                     start=(k == 0), stop=(k == EC - 1))
                nc.scalar.activation(badt[:, m:m + 1], ps[:, :], AF.Identity,
                                     bias=botS[:, m:m + 1])

            # --- step-0 hidden gates in bf16 (h0 ~ N(0,1): too large for
            # fp8 weight noise; later h is tanh-bounded) ---
            g0ps = ps3.tile([128, 2, 4, 2, PB], dt.float32, tag="pc")
            for k in range(EC):
                whhb = p1.tile([128, 16, 128], dt.bfloat16, tag="whhb", bufs=2)
                nc.scalar.dma_start(whhb[:, :, :],
                                    whhb_d[k * 128:(k + 1) * 128])
                for h in range(2):
                    for g in range(4):
                        for e2 in range(2):
                            nc.tensor.matmul(
                                g0ps[:, h, g, e2, :],
                                whhb[:, h * 8 + g * 2 + e2, :],
                                h0[:, k, :],
                                start=(k == 0 and h == 0 and g == 0 and e2 == 0),
                                stop=(k == EC - 1 and h == 1 and g == 3
                                      and e2 == 1))
            nc.any.tensor_copy(g0[:, :, :, :, :], g0ps[:, :, :, :, :])

            # ce (bulk) + oW now on the scalar queue
            for k in range(EC):
                nc.scalar.dma_start(
                    ce[:, k, :, :],
                    ceT_d[:, k * 128:(k + 1) * 128, :]
                    .rearrange("i p s -> p i s"))
            nc.scalar.dma_start(oW[:, :, :],
                                outWT_d.rearrange("(k p) n -> p k n", p=128))

            # --- te~^T[e, i, s] = M_t te^T ; beta[i, s] = v_t . te ---
            for m in range(EC):
                ps = cpp.tile([128, PB * STG], dt.float32, tag="cps")
                for k in range(EC):
                    nc.tensor.matmul(
                        ps[:, :], mtT[:, k, m * 128:(m + 1) * 128],
                        te[:, k, :, :].rearrange("p i s -> p (i s)"),
                        start=(k == 0), stop=(k == EC - 1))
                nc.any.tensor_copy(te2[:, m, :, :].rearrange("p i s -> p (i s)"),
                                   ps[:, :])
            bps = ps3.tile([1, PB * STG], dt.float32, tag="pc")
            for k in range(EC):
                nc.tensor.matmul(bps[:, :], vtC[:, k, :],
                                 te[:, k, :, :].rearrange("p i s -> p (i s)"),
                                 start=(k == 0), stop=(k == EC - 1))
            nc.any.tensor_copy(beta[:, :, :].rearrange("o i s -> o (i s)"),
                               bps[:, :])
            # --- V'_tag (replicated to 4 partition strips) ---
            for i in range(PB):
                ps = cpp.tile([STG, E], dt.float32, tag="cps")
                for k in range(EC):
                    nc.tensor.matmul(ps[:, :], te[:, k, i, :], wvot[:, k, :],
                                     start=(k == 0), stop=(k == EC - 1))
                nc.any.tensor_copy(vtp[:STG, i, :], ps[:, :])
            for di in range(1, 4):
                nc.sync.dma_start(vtp[di * STG:(di + 1) * STG, :, :],
                                  vtp[0:STG, :, :])
            # --- V'_char[s, i, g] ---
            for i in range(PB):
                for sc in range(SC // 128):
                    ps = cpp.tile([128, E], dt.float32, tag="cps")
                    for k in range(EC):
                        nc.tensor.matmul(
                            ps[:, :], ce[:, k, i, sc * 128:(sc + 1) * 128],
                            wvoc[:, k, :],
                            start=(k == 0), stop=(k == EC - 1))
                    nc.any.tensor_copy(vcp[:, sc, i, :], ps[:, :])

        # =================================================================
        # P3 unit builder (used interleaved for block 0, serial for block 1)
        # =================================================================
        p3 = ctx.enter_context(tc.tile_pool(name="p3", bufs=2))
        gtiles = {}

        def p3_group_units(blk, grp):
            t0 = blk * TB
            i0 = grp * GRP

            def u_qproj(m0):
                def f():
                    if m0 == 0:
                        qTg = p3.tile([128, EC, GRP, TB], dt.bfloat16,
                                      tag="qT", name="qTg")
                        org = p3.tile([128, 2 * EC, GRP, TB], dt.bfloat16,
                                      tag="og", name="org")
                        gtiles[(blk, grp)] = (qTg, org)
                    qTg, org = gtiles[(blk, grp)]
                    for m in (m0, m0 + 1):
                        ps = cpp.tile([128, TB, GRP], dt.float32, tag="cps")
                        for k in range(EC):
                            hk, ks = hch(k)
                            nc.tensor.matmul(
                                ps[:, :, :], mc[:, k, m, :],
                                hk[:, ks, t0:t0 + TB, i0:i0 + GRP],
                                start=(k == 0), stop=(k == EC - 1))
                        nc.vector.tensor_scalar(
                            qTg[:, m, :, :].rearrange("p i t -> p t i"),
                            ps[:, :, :], QE16, bqe[:, m:m + 1],
                            op0=AL.mult, op1=AL.add)
                return f

            def u_tag():
                qTg, org = gtiles[(blk, grp)]
                ptp = ps3.tile([128, GRP, STG], dt.float32, tag="pc")
                for di in range(GRP):
                    for k in range(EC):
                        hk, ks = hch(k)
                        nc.tensor.matmul(
                            ptp[:, di, :], hk[:, ks, t0:t0 + TB, i0 + di],
                            te2[:, k, i0 + di, :],
                            start=(di == 0 and k == 0), stop=False,
                            skip_group_check=True)
                nc.tensor.matmul(
                    ptp[:, :, :].rearrange("p i s -> p (i s)"),
                    ones1[:, :],
                    beta[:, i0:i0 + GRP, :].rearrange("o i s -> o (i s)"),
                    start=False, stop=True, skip_group_check=True)
                pte = p3.tile([128, GRP, STG], dt.bfloat16, tag="pte")
                nc.scalar.activation(pte[:, :, :], ptp[:, :, :], AF.Exp)
                tsum = p3.tile([128, GRP], dt.float32, tag="tsum")
                nc.vector.reduce_sum(tsum[:, :], pte[:, :, :], axis=AX.X)
                trec = p3.tile([128, GRP], dt.float32, tag="trec")
                nc.vector.reciprocal(trec[:, :], tsum[:, :])
                ptn = p3.tile([128, GRP, STG], dt.bfloat16, tag="ptn")
                for di in range(GRP):
                    nc.vector.tensor_scalar_mul(ptn[:, di, :], pte[:, di, :],
                                                trec[:, di:di + 1])
                ptT = p3.tile([128, TB], dt.bfloat16, tag="ptT")
                nc.sync.dma_start_transpose(
                    ptT[:, :], ptn[:, :, :].rearrange("p i s -> p (i s)"))
                gtiles[(blk, grp, "ptT")] = ptT

            def u_score(di):
                def f():
                    qTg, org = gtiles[(blk, grp)]
                    i = i0 + di
                    pc = ps3.tile([128, SC], dt.float32, tag="pc")
                    for k in range(EC):
                        nc.tensor.matmul(pc[:, :], qTg[:, k, di, :],
                                         ce[:, k, i, :],
                                         start=(k == 0), stop=(k == EC - 1))
                    pe = p3.tile([128, SC], dt.bfloat16, tag="pe")
                    dsum = p3.tile([128, 1], dt.float32, tag="dsum")
                    nc.scalar.activation(pe[:, :], pc[:, :], AF.Exp,
                                         accum_out=dsum[:, :])
                    drec = p3.tile([128, 1], dt.float32, tag="drec")
                    nc.vector.reciprocal(drec[:, :], dsum[:, :])
                    pn = p3.tile([128, SC], dt.bfloat16, tag="pn")
                    nc.vector.tensor_scalar_mul(pn[:, :], pe[:, :],
                                                drec[:, 0:1])
                    pTt = p3.tile([128, 2, TB], dt.bfloat16, tag="pTt")
                    for sc in range(2):
                        nc.sync.dma_start_transpose(
                            pTt[:, sc, :], pn[:, sc * 128:(sc + 1) * 128])
                    gtiles[(blk, grp, di)] = pTt
                return f

            def u_ctx(di):
                def f():
                    qTg, org = gtiles[(blk, grp)]
                    pTt = gtiles[(blk, grp, di)]
                    ptT = gtiles[(blk, grp, "ptT")]
                    i = i0 + di
                    cps = cpp.tile([128, EC, TB], dt.float32, tag="cps")
                    for m in range(EC):
                        for sc in range(2):
                            nc.tensor.matmul(
                                cps[:, m, :],
                                vcp[:, sc, i, m * 128:(m + 1) * 128],
                                pTt[:, sc, :],
                                start=(m == 0 and sc == 0),
                                stop=(m == EC - 1 and sc == 1))
                    for m in range(EC):
                        nc.vector.tensor_scalar(
                            org[:, m, di, :], cps[:, m, :],
                            badc[:, m:m + 1], 0.0, op0=AL.add, op1=AL.max)
                    cp2 = cpp.tile([128, EC, TB], dt.float32, tag="cps")
                    for m in range(EC):
                        nc.tensor.matmul(
                            cp2[:, m, :],
                            vtp[di * STG:(di + 1) * STG, i,
                                m * 128:(m + 1) * 128],
                            ptT[di * STG:(di + 1) * STG, :],
                            start=(m == 0), stop=(m == EC - 1),
                            tile_position=(di * STG, 0))
                    for m in range(EC):
                        nc.vector.tensor_scalar(
                            org[:, EC + m, di, :], cp2[:, m, :],
                            badt[:, m:m + 1], 0.0, op0=AL.add, op1=AL.max)
                return f

            def u_out():
                qTg, org = gtiles[(blk, grp)]
                ps = cpp.tile([128, GRP, TB], dt.float32, tag="cps")
                for kk in range(2 * EC):
                    nc.tensor.matmul(
                        ps[:, :, :].rearrange("p i t -> p (i t)"), oW[:, kk, :],
                        org[:, kk, :, :].rearrange("p i t -> p (i t)"),
                        start=(kk == 0), stop=(kk == 2 * EC - 1))
                of = p3.tile([128, GRP, TB], dt.float32, tag="of")
                nc.vector.tensor_scalar_add(
                    of[:, :, :].rearrange("p i t -> p (i t)"),
                    ps[:, :, :].rearrange("p i t -> p (i t)"), ob[:, 0:1])
                nc.sync.dma_start(
                    out_d[i0:i0 + GRP, :, t0:t0 + TB]
                    .rearrange("i n t -> n i t"),
                    of[:, :, :])

            units = [u_qproj(0), u_qproj(2), u_tag]
            for di in range(GRP):
                units.append(u_score(di))
                units.append(u_ctx(di))
            units.append(u_out)
            return units

        def p3_block_tail(blk):
            # pairwise group interleave: hides the DMA-transpose latency
            # between a sample's score and ctx units while keeping only two
            # groups' tiles live (pool bufs=2)
            for gp0 in range(0, PB // GRP, 2):
                ua = p3_group_units(blk, gp0)
                ub = p3_group_units(blk, gp0 + 1)
                for a, b in zip(ua, ub):
                    a()
                    b()

        # =================================================================
        # P2: sequential LSTM with interleaved filler work
        # =================================================================
        fillq = []
        for grp in range(PB // GRP):
            fillq.extend(p3_group_units(0, grp))

        for t in range(Ts):
            if t % SLAB == 0:
                v = t // SLAB
                if v + 2 < NW:
                    load_xbuf(v + 2)
            slab = slabs[t // SLAB]
            for h in range(2):
                gph = None
                if t > 0:
                    gph = gpp.tile([128, 4, 2, PB], dt.float32, tag=f"gp{h}")
                    for k in range(EC):
                        hk, ks = hch(k)
                        rhs = hk[:, ks, t - 1, :]
                        for g in range(4):
                            for e2 in range(2):
                                nc.tensor.matmul(
                                    gph[:, g, e2, :],
                                    whh[:, k, h * 8 + g * 2 + e2, :], rhs,
                                    start=(k == 0 and g == 0 and e2 == 0),
                                    stop=(k == EC - 1 and g == 3 and e2 == 1))
                gsrc = gph if t > 0 else g0[:, h]
                ga = p2w.tile([128, 4, 2, PB], dt.float32, tag=f"ga{h}")
                nc.vector.tensor_add(ga[:, :, :, :], gsrc[:, :, :, :],
                                     slab[:, t % SLAB, h, :, :, :])
                # g-gate rows are pre-scaled x2 host-side, so one sigmoid
                # covers all four gates: tanh(g) = 2*sigmoid(2g) - 1
                sio = p2w.tile([128, 4, 2, PB], dt.float32, tag=f"sio{h}")
                nc.scalar.activation(sio[:, :, :, :], ga[:, :, :, :],
                                     AF.Sigmoid)
                v_ = p2w.tile([128, 2, PB], dt.float32, tag=f"v{h}")
                a_ = p2w.tile([128, 2, PB], dt.float32, tag=f"a{h}")
                nc.vector.tensor_mul(v_[:, :, :], sio[:, 1, :, :],
                                     cT[:, 2 * h:2 * h + 2, :])
                # a = (sig(2g) - 0.5) * sig(i) = i*tanh(g)/2
                nc.vector.scalar_tensor_tensor(
                    a_[:, :, :], sio[:, 3, :, :], 0.5, sio[:, 0, :, :],
                    op0=AL.subtract, op1=AL.mult)
                nc.vector.scalar_tensor_tensor(
                    cT[:, 2 * h:2 * h + 2, :], a_[:, :, :], 2.0, v_[:, :, :],
                    op0=AL.mult, op1=AL.add)
                tcc = p2w.tile([128, 2, PB], dt.float32, tag=f"tcc{h}")
                nc.scalar.activation(tcc[:, :, :],
                                     cT[:, 2 * h:2 * h + 2, :], AF.Tanh)
                # hT stores h/16 (compensates the 16x fp8 Whh scaling)
                nc.vector.scalar_tensor_tensor(
                    (hTa if h == 0 else hTb)[:, :, t, :], sio[:, 2, :, :],
                    1.0 / 16.0, tcc[:, :, :], op0=AL.mult, op1=AL.mult)
            # filler: next slab window's XWT tile; P3 block-0 units once
            # their h block is complete
            if t < Ts - SLAB:
                xwt_unit(t // SLAB + 1, t % SLAB)
            if t >= TB + 2 and fillq:
                fillq.pop(0)()

        while fillq:
            fillq.pop(0)()
        for blk in range(1, NBLK):
            p3_block_tail(blk)

    nc.compile()
    return nc


def _prep_core(inputs, core, Ts=T):
    bf = ml_dtypes.bfloat16
    f8 = ml_dtypes.float8_e3m4
    s = slice(core * PB, (core + 1) * PB)
    ce = inputs["char_encoding"][s]
    teg = inputs["tag_encoding"][s]
    tos = inputs["true_output_seq"][s][:, :Ts]
    xs = np.concatenate(
        [np.zeros((PB, 1, NCH), np.float32), tos[:, 1:, :]], axis=1)
    # Whh/Wih rows: torch gate order (i,f,g,o) -> (i,f,o,g); feature chunk
    # ec split as (half, e2); tile j = half*8 + gt*2 + e2.
    # g-gate rows x2: tanh(g) computed as 2*sigmoid(2g) - 1 on device
    W = inputs["lstm_Whh"].reshape(4, 4, 128, E)[[0, 1, 3, 2]].copy()
    W[3] *= 2.0
    whhP = W.reshape(4, 2, 2, 128, E).transpose(4, 1, 0, 2, 3).reshape(E, 16, 128)
    V = inputs["lstm_Wih"].reshape(4, 4, 128, NCH)[[0, 1, 3, 2]].copy()
    V[3] *= 2.0
    wihP = V.reshape(4, 2, 2, 128, NCH).transpose(4, 1, 0, 2, 3).reshape(NCH, 16, 128)
    gbv = (inputs["lstm_bih"] + inputs["lstm_bhh"]).reshape(4, 4, 128)[[0, 1, 3, 2]].copy()
    gbv[3] *= 2.0
    gbias = gbv.reshape(4, 2, 2, 128).transpose(1, 0, 2, 3).reshape(16, 128)
    m = {
        "ceT": np.ascontiguousarray(ce.transpose(0, 2, 1)).astype(bf),
        "teT": np.ascontiguousarray(teg.transpose(0, 2, 1)).astype(bf),
        "xT": np.ascontiguousarray(xs.transpose(2, 1, 0)).astype(bf),
        "whhP": np.ascontiguousarray(whhP * 16.0).astype(f8),
        "whhB": np.ascontiguousarray(whhP).astype(bf),
        "wihP": np.ascontiguousarray(wihP).astype(bf),
        "gbias": np.ascontiguousarray(gbias).astype(np.float32),
        "wqc": inputs["ca_Wq"].astype(bf),
        "wkc": inputs["ca_Wk"].astype(bf),
        "wvc": inputs["ca_Wv"].astype(bf),
        "wocT": np.ascontiguousarray(inputs["ca_Wo"].T).astype(bf),
        "wqt": inputs["ta_Wq"].astype(bf),
        "wkt": inputs["ta_Wk"].astype(bf),
        "wvt": inputs["ta_Wv"].astype(bf),
        "wotT": np.ascontiguousarray(inputs["ta_Wo"].T).astype(bf),
        "bqc_col": inputs["ca_bq"][:, None].astype(bf),
        "bvc_col": inputs["ca_bv"][:, None].astype(bf),
        "bqt_col": inputs["ta_bq"][:, None].astype(bf),
        "bvt_col": inputs["ta_bv"][:, None].astype(bf),
        "boc": inputs["ca_bo"].astype(np.float32),
        "bot": inputs["ta_bo"].astype(np.float32),
        "outWT": np.ascontiguousarray(inputs["out_W"].T).astype(bf),
        "outb": inputs["out_b"].astype(np.float32),
        "h0T": np.ascontiguousarray(
            np.concatenate([inputs["char_hn"][0][s],
                            inputs["char_hn"][1][s]], -1).T).astype(bf),
        "c0T": np.ascontiguousarray(
            np.concatenate([inputs["char_cn"][0][s],
                            inputs["char_cn"][1][s]], -1).T).astype(np.float32),
    }
    return m


def kernel(**inputs):
    from concourse.bass_utils import run_bass_kernel_spmd

    inputs = {k: np.asarray(v, dtype=np.float32) for k, v in inputs.items()}
    if "nc" not in _cache:
        _cache["nc"] = _build(T)
    nc = _cache["nc"]
    in_maps = [_prep_core(inputs, c) for c in range(NCORES)]
    res = run_bass_kernel_spmd(nc, in_maps, list(range(NCORES)))
    _cache["last_res"] = res
    outs = [np.asarray(res.results[c]["out"]).transpose(0, 2, 1)
            for c in range(NCORES)]
    return np.ascontiguousarray(np.concatenate(outs, axis=0)).astype(np.float32)
